# revision 40
# baseline (speedup 1.0000x reference)
"""Trainium2 Bass kernel for nn_Decoder_60198261621363.

6-layer dense transformer decoder (E=1024, H=16, FF=4096, V=32000, B=4, T=1024).

Sharding: data-parallel over tokens across 8 cores. Core c owns batch c//2,
token half c%2 (512 tokens). Weights are replicated (bf16); each layer does a
pairwise AllGather carrying that layer's K^T and V so the paired core sees the
other half's keys/values. Activations are feature-major (x^T: [E, tokens]);
the residual stream stays f32, all matmul operands are bf16.

Attention is two-pass to hide the AllGather: pass 1 computes own-half scores
and exps for ALL heads (suffix-restricted to the causal region, stored in a
persistent SBUF tile), pass 2 (first consumer of partner K/V) does partner
scores plus the full AV accumulation per head, with a 2-head score lookahead
so the partner-V readback is hidden too.  Own-half key chunks only need the
causal suffix of queries (plus a 128x128 triangular multiply on the diagonal
block); partner-half chunks are fully allowed or fully masked via a per-core
exp bias of 0 / -1e30.

LayerNorm: the gains are folded into the weights on the host (rows of
Wq/Wk/Wv scaled by ln1_g, W1 by ln2_g; ln biases are zero in this model), so
the kernel only computes hu = x - m on the h critical path and applies the
per-token 1/std at psum evacuation time (a column scale for feature-major
outputs; a per-partition scale via a transposed rT column for the token-major
V output).

All streamed weights are pre-transposed on the host into [128, k*free]
layouts so each partition reads one contiguous run per DMA.
"""
import sys

sys.path.insert(0, "/opt/trn_rl_repo")

import ml_dtypes
import numpy as np

import concourse.bass as bass
import concourse.mybir as mybir
import concourse.tile as tile
from concourse import bacc
from concourse.bass_utils import run_bass_kernel_spmd
from concourse.masks import make_identity

AF = mybir.ActivationFunctionType
ALU = mybir.AluOpType
F32 = mybir.dt.float32
F32R = mybir.dt.float32r
BF16 = mybir.dt.bfloat16
FP8 = mybir.dt.float8e4
I32 = mybir.dt.int32
NPBF16 = ml_dtypes.bfloat16

V, E, L, H, B, T = 32000, 1024, 6, 16, 4, 1024
HS = E // H          # 64
FF = 4 * E           # 4096
NC = 8               # cores
RT = 512             # tokens per core
KT = E // 128        # 8  E-tiles
TT = RT // 128       # 4  token-tiles per core
VCH = 1000           # lm-head column chunk (2 psum groups of 500)
NVC = V // VCH       # 32 chunks
SCALE = float(E) ** -0.5

# own-half score suffix: chunk kcc covers keys kcc*128..; only queries
# >= kcc*128 can see them.  eo column offsets pack the suffixes.
SUF = [RT - kcc * 128 for kcc in range(TT)]        # 512,384,256,128
EOFF = [0]
for w in SUF:
    EOFF.append(EOFF[-1] + w)
EOW = EOFF[-1]                                      # 1280

_cache = {}


def _build(n_layers=L, dbg=False):
    nc = bacc.Bacc("TRN2", target_bir_lowering=False, debug=False, num_devices=NC)
    dtensors = {}
    if dbg:
        for nm, shp, dt in [
            ("dbg_xemb", [128, KT, RT], F32), ("dbg_h1", [128, KT, RT], BF16),
            ("dbg_kown", [128, KT, RT], BF16), ("dbg_kpart", [128, KT, RT], BF16),
            ("dbg_vaug", [128, TT, H, HS + 1], BF16),
            ("dbg_vpaug", [128, TT, H, HS + 1], BF16),
            ("dbg_qT", [128, KT, RT], BF16),
            ("dbg_aT", [128, KT, RT], BF16), ("dbg_xl0", [128, KT, RT], F32),
        ]:
            dtensors[nm] = nc.dram_tensor(nm, shp, dt, kind="ExternalOutput")

    tok_emb = nc.dram_tensor("tok_emb", [V, E], BF16, kind="ExternalInput")
    tok_idx = nc.dram_tensor("tok_idx", [128, TT], I32, kind="ExternalInput")
    gidx = nc.dram_tensor("gidx", [128, 2], I32, kind="ExternalInput")
    posT = nc.dram_tensor("posT", [128, KT * RT], BF16, kind="ExternalInput")
    trid = nc.dram_tensor("trid", [128, 128], BF16, kind="ExternalInput")
    pbiasd = nc.dram_tensor("pbiasd", [128, 1], F32, kind="ExternalInput")
    Wq = nc.dram_tensor("Wq", [n_layers, 128, KT * E], BF16, kind="ExternalInput")
    Wk = nc.dram_tensor("Wk", [n_layers, 128, KT * E], BF16, kind="ExternalInput")
    Wv = nc.dram_tensor("Wv", [n_layers, 128, KT * E], BF16, kind="ExternalInput")
    Wo = nc.dram_tensor("Wo", [n_layers, 128, KT * E], BF16, kind="ExternalInput")
    W1 = nc.dram_tensor("W1", [n_layers, 4, 128, KT * E], BF16,
                        kind="ExternalInput")
    W2 = nc.dram_tensor("W2", [n_layers, 2, 2, 128, 16 * 512], BF16,
                        kind="ExternalInput")
    b1d = nc.dram_tensor("b1", [n_layers, 128, FF // 128], F32, kind="ExternalInput")
    b2d = nc.dram_tensor("b2", [n_layers, 128, KT], F32, kind="ExternalInput")
    lmhw = nc.dram_tensor("lmhw", [NVC, 128, KT * VCH], BF16, kind="ExternalInput")
    lmhb = nc.dram_tensor("lmhb", [128, V], BF16, kind="ExternalInput")
    logits = nc.dram_tensor("logits", [RT, V], BF16, kind="ExternalOutput")

    rgroups = [[0, 1], [2, 3], [4, 5], [6, 7]]
    AGK = E * RT        # elements of K^T in the AG payload
    AUGW = H * (HS + 1)                 # 1040 head-blocked V cols per tt
    AGV = TT * AUGW                     # per-partition V payload cols

    with tile.TileContext(nc) as tc:
        with (
            tc.tile_pool(name="persist", bufs=1) as pp,
            tc.tile_pool(name="sb", bufs=2) as sb,
            tc.tile_pool(name="wstream", bufs=2) as wp,
            tc.tile_pool(name="psm", bufs=3, space="PSUM") as psm,   # dense+scores
            tc.tile_pool(name="psa", bufs=3, space="PSUM") as psa,   # attn out
            tc.tile_pool(name="pst", bufs=1, space="PSUM") as pst,   # ln stats
            tc.tile_pool(name="dram", bufs=2, space="DRAM") as dram,
        ):
            # ---- persistent tiles
            x = pp.tile([128, KT, RT], F32R)       # residual stream x^T
            ones = pp.tile([128, 128], F32R)
            ident = pp.tile([128, 128], BF16)
            identf = pp.tile([128, 128], F32)
            make_identity(nc, identf[:])
            epsb = pp.tile([128, 1], F32)
            tri = pp.tile([128, 128], BF16)        # lower-tri (key<=query)
            pbias = pp.tile([128, 1], F32)
            nc.vector.memset(ones[:].bitcast(F32), 1.0)
            nc.vector.memset(epsb[:], 1e-5)
            make_identity(nc, ident[:])
            nc.sync.dma_start(tri[:], trid[:])
            nc.sync.dma_start(pbias[:], pbiasd[:])
            gidx_sb = pp.tile([128, 2], I32)
            nc.sync.dma_start(gidx_sb[:], gidx[:])

            v_aug = pp.tile([128, TT, H, HS + 1], BF16)   # own V, head-blocked
            vp_aug = pp.tile([128, TT, H, HS + 1], BF16)  # partner V
            nc.vector.memset(v_aug[:, :, :, HS:HS + 1], 1.0)
            nc.vector.memset(vp_aug[:, :, :, HS:HS + 1], 1.0)

            # pass-1 own-half attention accumulators (numerator rows 0-63
            # of each head block + denominator row 64), spilled from PSUM
            anum = pp.tile([128, H, RT], BF16)
            # zero-padded qT variants: qTz0 keeps even-head rows (0-63 of
            # each chunk), qTz1 keeps odd-head rows; the other half is zero
            # so score matmuls can use the full 128-row K^T block as a
            # dense-shaped stationary (FWL + weight-load pipelining).
            qTz0 = pp.tile([128, KT, RT], BF16)
            qTz1 = pp.tile([128, KT, RT], BF16)
            nc.vector.memset(qTz0[64:128, :, :], 0.0)
            nc.vector.memset(qTz1[0:64, :, :], 0.0)

            # ---- embedding gather + transpose + pos add
            idx = pp.tile([128, TT], I32)
            nc.sync.dma_start(idx[:], tok_idx[:])
            pos_sb = wp.tile([128, KT, RT], BF16, tag="wstream")
            nc.sync.dma_start(
                pos_sb[:], posT.ap().rearrange("p (k t) -> p k t", t=RT))
            for g in range(TT):
                emb = sb.tile([128, E], BF16, tag="emb", bufs=1)
                nc.gpsimd.indirect_dma_start(
                    out=emb[:], out_offset=None, in_=tok_emb[:],
                    in_offset=bass.IndirectOffsetOnAxis(
                        ap=idx[:, g:g + 1], axis=0),
                )
                for kt in range(KT):
                    tp = psm.tile([128, 128], BF16, space="PSUM", tag="psm")
                    nc.tensor.transpose(tp[:], emb[:, kt * 128:(kt + 1) * 128],
                                        ident[:])
                    nc.vector.tensor_add(
                        out=x[:, kt, g * 128:(g + 1) * 128],
                        in0=tp[:],
                        in1=pos_sb[:, kt, g * 128:(g + 1) * 128])

            if dbg:
                nc.sync.dma_start(dtensors["dbg_xemb"].ap(), x[:].bitcast(F32))

            def layer_norm():
                """Returns (hu, r, None): hu = x - mean (bf16, the matmul
                operand; LN gain is folded into the weights on the host),
                r = 1/std as a row-broadcast [128, RT] f32 (column scale for
                feature-major psum outputs)."""
                sum_ps = pst.tile([128, RT], F32, space="PSUM", tag="pst")
                sq_ps = pst.tile([128, RT], F32, space="PSUM", tag="pst2")
                for kt in range(KT):
                    sq = sb.tile([128, RT], F32R, tag="lnsq")
                    nc.scalar.activation(sq[:], x[:, kt, :], AF.Square)
                    nc.tensor.matmul(sum_ps[:], ones[:], x[:, kt, :],
                                     start=(kt == 0), stop=(kt == KT - 1))
                    nc.tensor.matmul(sq_ps[:], ones[:], sq[:],
                                     start=(kt == 0), stop=(kt == KT - 1))
                m = sb.tile([128, RT], F32, tag="ln_m", bufs=1)
                nc.vector.tensor_scalar_mul(m[:], sum_ps[:], 1.0 / E)
                hu = sb.tile([128, KT, RT], BF16, tag="h", bufs=1)
                for kt in range(KT):
                    nc.vector.tensor_sub(hu[:, kt, :], x[:, kt, :], m[:])
                mm = sb.tile([128, RT], F32, tag="ln_t")
                nc.vector.tensor_mul(mm[:], m[:], m[:])
                var = sb.tile([128, RT], F32, tag="ln_t")
                nc.vector.scalar_tensor_tensor(
                    var[:], sq_ps[:], 1.0 / E, mm[:], ALU.mult, ALU.subtract)
                std = sb.tile([128, RT], F32, tag="ln_t")
                nc.scalar.activation(std[:], var[:], AF.Sqrt, bias=epsb[:, 0:1])
                r = sb.tile([128, RT], F32, tag="ln_r", bufs=1)
                nc.vector.reciprocal_approx_fast(r[:], std[:])
                return hu, r, None

            def wload(src_ap, shape, tag="wstream"):
                wt = wp.tile([128] + shape, BF16, tag=tag)
                nc.sync.dma_start(
                    wt[:], src_ap.rearrange("p (k m) -> p k m", m=shape[-1]))
                return wt

            for li in range(n_layers):
                # ================= attention =================
                hu, r, _ = layer_norm()

                agin_k = dram.tile([AGK], BF16, tag="agin_k")
                agout_k = dram.tile([2, AGK], BF16, tag="agout_k")
                agin_v = dram.tile([128 * AGV], BF16, tag="agin_v")
                agout_v = dram.tile([2, 128 * AGV], BF16, tag="agout_v")

                # K^T own  [E, RT] as [128, KT, RT]; columns scaled by r
                kT_own = sb.tile([128, KT, RT], BF16, tag="kT_own", bufs=1)
                wt = wload(Wk[li], [KT, E])
                for mt in range(KT):
                    ps = psm.tile([128, RT], F32, space="PSUM", tag="psm")
                    for kt in range(KT):
                        nc.tensor.matmul(
                            ps[:], wt[:, kt, mt * 128:(mt + 1) * 128],
                            hu[:, kt, :],
                            start=(kt == 0), stop=(kt == KT - 1))
                    nc.vector.scalar_tensor_tensor(
                        kT_own[:, mt, :], ps[:], 0.0, r[:],
                        ALU.bypass, ALU.mult)

                nc.sync.dma_start(
                    agin_k[:].rearrange("(f t) -> f t", t=RT)
                    .rearrange("(k p) t -> p k t", p=128), kT_own[:])
                nc.gpsimd.collective_compute(
                    "AllGather", mybir.AluOpType.bypass,
                    replica_groups=rgroups,
                    ins=[agin_k[:].opt()], outs=[agout_k[:].opt()],
                )

                # rT for the token-major V evac (r is ready by now; placed
                # here so the transposes don't block the K matmuls)
                rT = sb.tile([128, TT], F32, tag="ln_rT", bufs=1)
                for tt in range(TT):
                    tp = psm.tile([128, 128], F32, space="PSUM", tag="psm")
                    nc.tensor.transpose(
                        tp[:], r[:, tt * 128:(tt + 1) * 128], identf[:])
                    nc.vector.tensor_copy(rT[:, tt:tt + 1], tp[:, 0:1])

                # V own, projected straight into the head-blocked aug layout;
                # rows (tokens) scaled by rT
                wt = wload(Wv[li], [KT, E])
                for tt in range(TT):
                    for mh in range(2):
                        ps = psm.tile([128, RT], F32, space="PSUM", tag="psm")
                        for kt in range(KT):
                            nc.tensor.matmul(
                                ps[:], hu[:, kt, tt * 128:(tt + 1) * 128],
                                wt[:, kt, mh * 512:(mh + 1) * 512],
                                start=(kt == 0), stop=(kt == KT - 1))
                        nc.vector.tensor_scalar_mul(
                            v_aug[:, tt, mh * 8:(mh + 1) * 8, 0:HS],
                            ps[:].rearrange("p (h f) -> p h f", f=HS),
                            rT[:, tt:tt + 1])

                # AG payload is the aug layout itself (ones column included)
                nc.sync.dma_start(
                    agin_v[:].rearrange("(p f) -> p f", p=128),
                    v_aug[:])
                nc.gpsimd.collective_compute(
                    "AllGather", mybir.AluOpType.bypass,
                    replica_groups=rgroups,
                    ins=[agin_v[:].opt()], outs=[agout_v[:].opt()],
                )

                # Q^T; columns scaled by r, split into the zero-padded
                # even/odd-head variants
                wt = wload(Wq[li], [KT, E])
                for mt in range(KT):
                    ps = psm.tile([128, RT], F32, space="PSUM", tag="psm")
                    for kt in range(KT):
                        nc.tensor.matmul(
                            ps[:], wt[:, kt, mt * 128:(mt + 1) * 128],
                            hu[:, kt, :],
                            start=(kt == 0), stop=(kt == KT - 1))
                    nc.vector.scalar_tensor_tensor(
                        qTz0[0:64, mt, :], ps[0:64, :], 0.0, r[0:64, :],
                        ALU.bypass, ALU.mult)
                    nc.vector.scalar_tensor_tensor(
                        qTz1[64:128, mt, :], ps[64:128, :], 0.0, r[64:128, :],
                        ALU.bypass, ALU.mult)

                # ---- pass 1: own-half attention for ALL heads: scores
                # (full 128-row K^T block as stationary, zero-padded qTz as
                # moving), exp (+ triangular mask on the diagonal block),
                # own-half AV accumulation, spilled to anum (numerator in
                # rows 0-63 of the head block, denominator in row 64).
                # Chunk kcc only affects queries >= kcc*128 (suffix).
                # Gives the AllGathers a long window to hide.
                for hd in range(H):
                    hp, sub = hd // 2, hd % 2
                    qz = qTz0 if sub == 0 else qTz1
                    es_ = []
                    for kcc in range(TT):
                        c0 = kcc * 128
                        sc = psm.tile([128, RT], F32, space="PSUM", tag="psm")
                        nc.tensor.matmul(
                            sc[:, c0:],
                            kT_own[:, hp, c0:c0 + 128],
                            qz[:, hp, c0:],
                            start=True, stop=True)
                        es = sb.tile([128, RT], BF16, tag="esc", bufs=12)
                        nc.scalar.activation(es[:, c0:], sc[:, c0:], AF.Exp,
                                             scale=SCALE)
                        nc.vector.tensor_mul(
                            es[:, c0:c0 + 128], es[:, c0:c0 + 128], tri[:])
                        es_.append(es)
                    av = psa.tile([128, RT], F32, space="PSUM", tag="psa")
                    for kcc in range(TT):
                        c0 = kcc * 128
                        nc.tensor.matmul(
                            av[0:HS + 1, c0:], v_aug[:, kcc, hd, :],
                            es_[kcc][:, c0:],
                            start=(kcc == 0), stop=(kcc == TT - 1))
                    nc.vector.tensor_copy(anum[0:HS + 1, hd, :],
                                          av[0:HS + 1, :])

                # partner K^T / V readback (depends on the AGs)
                kflat = agout_k[:].rearrange("s (f t) -> (s f) t", t=RT)
                # rows of AUGW so the indexed row length matches the
                # per-partition transfer length (as for K)
                vflat = agout_v[:].rearrange("s (p t f) -> (s p t) f",
                                             f=AUGW, t=TT)
                kT_part = sb.tile([128, KT, RT], BF16, tag="kT_part", bufs=1)
                for mt in range(KT):
                    nc.gpsimd.indirect_dma_start(
                        out=kT_part[:, mt, :], out_offset=None,
                        in_=kflat,
                        in_offset=bass.IndirectOffsetOnAxis(
                            ap=gidx_sb[:, 0:1], axis=0),
                        element_offset=mt * 128 * RT,
                    )
                # NOTE: the indirect offset is effectively scaled by the
                # dest AP's innermost contiguous run length, so the dest
                # must be flattened to rows of AUGW (matching gidx, which
                # holds pshard*512 + p*TT row indices).
                for tt in range(TT):
                    nc.gpsimd.indirect_dma_start(
                        out=vp_aug[:, tt, :, :].rearrange("p h f -> p (h f)"),
                        out_offset=None,
                        in_=vflat,
                        in_offset=bass.IndirectOffsetOnAxis(
                            ap=gidx_sb[:, 1:2], axis=0),
                        element_offset=tt * AUGW,
                    )

                # ---- pass 2: per head, partner scores (full width, exp
                # bias 0 / -1e30 per core), partner AV accumulation, then
                # combine with the pass-1 own-half accumulators.  2-head
                # score lookahead hides the partner-V readback.
                aT = sb.tile([128, KT, RT], BF16, tag="aT", bufs=1)
                p2eps = {}

                def p2_scores(hd):
                    hp, sub = hd // 2, hd % 2
                    qz = qTz0 if sub == 0 else qTz1
                    eps_ = []
                    for kcc in range(TT):
                        sc = psm.tile([128, RT], F32, space="PSUM", tag="psm")
                        nc.tensor.matmul(
                            sc[:],
                            kT_part[:, hp, kcc * 128:(kcc + 1) * 128],
                            qz[:, hp, :],
                            start=True, stop=True)
                        ep = sb.tile([128, RT], BF16, tag="esc", bufs=12)
                        nc.scalar.activation(ep[:], sc[:], AF.Exp,
                                             scale=SCALE, bias=pbias[:, 0:1])
                        eps_.append(ep)
                    p2eps[hd] = eps_

                def p2_avs(hd):
                    hp, lo = hd // 2, (hd % 2) * 64
                    eps_ = p2eps.pop(hd)
                    av = psa.tile([128, RT], F32, space="PSUM", tag="psa")
                    for kcc in range(TT):
                        nc.tensor.matmul(
                            av[0:HS + 1, :], vp_aug[:, kcc, hd, :],
                            eps_[kcc][:],
                            start=(kcc == 0), stop=(kcc == TT - 1))
                    num = sb.tile([64, RT], F32, tag="numt", bufs=2)
                    nc.vector.tensor_add(num[:], av[0:HS, :],
                                         anum[0:HS, hd, :])
                    dtmp = sb.tile([1, RT], F32, tag="dtmp", bufs=2)
                    nc.vector.tensor_add(dtmp[0:1, :], av[HS:HS + 1, :],
                                         anum[HS:HS + 1, hd, :])
                    nc.vector.reciprocal_approx_fast(dtmp[0:1, :],
                                                     dtmp[0:1, :])
                    rtmp = sb.tile([64, RT], F32, tag="rtmp", bufs=2)
                    nc.gpsimd.partition_broadcast(rtmp[:], dtmp[0:1, :])
                    nc.gpsimd.tensor_mul(aT[lo:lo + 64, hp, :], num[:],
                                         rtmp[:])

                LOOK = 2
                for hd in range(H + LOOK):
                    if hd < H:
                        p2_scores(hd)
                    if hd >= LOOK:
                        p2_avs(hd - LOOK)

                if dbg and li == 0:
                    nc.sync.dma_start(dtensors["dbg_h1"].ap(), hu[:])
                    nc.sync.dma_start(dtensors["dbg_kown"].ap(), kT_own[:])
                    nc.sync.dma_start(dtensors["dbg_kpart"].ap(), kT_part[:])
                    nc.sync.dma_start(dtensors["dbg_vaug"].ap(), v_aug[:])
                    nc.sync.dma_start(dtensors["dbg_vpaug"].ap(), vp_aug[:])
                    nc.sync.dma_start(dtensors["dbg_qT"].ap(), qTz0[:])
                    nc.sync.dma_start(dtensors["dbg_aT"].ap(), aT[:])

                # ---- Wo + residual
                wt = wload(Wo[li], [KT, E])
                for mt in range(KT):
                    ps = psm.tile([128, RT], F32, space="PSUM", tag="psm")
                    for kt in range(KT):
                        nc.tensor.matmul(
                            ps[:], wt[:, kt, mt * 128:(mt + 1) * 128],
                            aT[:, kt, :],
                            start=(kt == 0), stop=(kt == KT - 1))
                    nc.vector.tensor_add(x[:, mt, :], x[:, mt, :], ps[:])

                # ================= FFN (two halves of FF) =================
                hu2, r2, _ = layer_norm()
                b1t = sb.tile([128, FF // 128], F32, tag="b1t")
                nc.sync.dma_start(b1t[:], b1d[li])
                b2t = sb.tile([128, KT], F32, tag="b2t")
                nc.sync.dma_start(b2t[:], b2d[li])
                for fh in range(2):
                    up = sb.tile([128, 16, RT], BF16, tag="up", bufs=1)
                    for c in range(2):
                        wt = wload(W1[li, fh * 2 + c], [KT, E])
                        for mt in range(KT):
                            ps = psm.tile([128, RT], F32, space="PSUM", tag="psm")
                            for kt in range(KT):
                                nc.tensor.matmul(
                                    ps[:], wt[:, kt, mt * 128:(mt + 1) * 128],
                                    hu2[:, kt, :],
                                    start=(kt == 0), stop=(kt == KT - 1))
                            # scale by 1/std in place, then biased relu
                            nc.vector.scalar_tensor_tensor(
                                ps[:], ps[:], 0.0, r2[:], ALU.bypass, ALU.mult)
                            gft = (fh * 2 + c) * 8 + mt
                            nc.scalar.activation(up[:, c * 8 + mt, :], ps[:],
                                                 AF.Relu, bias=b1t[:, gft:gft + 1])
                    for m2 in range(2):
                        wt = wload(W2[li, fh, m2], [16, 512])
                        for mt in range(4):
                            ps = psm.tile([128, RT], F32, space="PSUM", tag="psm")
                            for kt in range(16):
                                nc.tensor.matmul(
                                    ps[:], wt[:, kt, mt * 128:(mt + 1) * 128],
                                    up[:, kt, :],
                                    start=(kt == 0), stop=(kt == 15))
                            ft = m2 * 4 + mt
                            if fh == 1:
                                nc.vector.scalar_tensor_tensor(
                                    x[:, ft, :], ps[:], b2t[:, ft:ft + 1],
                                    x[:, ft, :],
                                    ALU.add, ALU.add)
                            else:
                                nc.vector.tensor_add(x[:, ft, :], x[:, ft, :],
                                                     ps[:])
                if dbg and li == 0:
                    nc.sync.dma_start(dtensors["dbg_xl0"].ap(), x[:].bitcast(F32))

            # ================= LM head =================
            xb = sb.tile([128, KT, RT], BF16, tag="h", bufs=1)
            for kt in range(KT):
                nc.vector.tensor_copy(xb[:, kt, :], x[:, kt, :])
            for vc in range(NVC):
                wt = wload(lmhw[vc], [KT, VCH])
                bb = sb.tile([128, VCH], BF16, tag="bb")
                nc.sync.dma_start(bb[:], lmhb[:, vc * VCH:(vc + 1) * VCH])
                for tt in range(TT):
                    lg = sb.tile([128, VCH], BF16, tag="lg", bufs=3)
                    for hv in range(2):
                        ps = psm.tile([128, VCH // 2], F32, space="PSUM",
                                      tag="psm")
                        for kt in range(KT):
                            nc.tensor.matmul(
                                ps[:], xb[:, kt, tt * 128:(tt + 1) * 128],
                                wt[:, kt, hv * 500:(hv + 1) * 500],
                                start=(kt == 0), stop=(kt == KT - 1))
                        nc.vector.tensor_add(
                            lg[:, hv * 500:(hv + 1) * 500], ps[:],
                            bb[:, hv * 500:(hv + 1) * 500])
                    nc.sync.dma_start(
                        logits.ap().rearrange("(t p) v -> p t v", p=128)
                        [:, tt, vc * VCH:(vc + 1) * VCH], lg[:])
    nc.compile()
    return nc


def _prepare(inputs, n_layers=L):
    """Build the 8 per-core input maps from full inputs."""
    f = lambda a: np.ascontiguousarray(np.asarray(a), dtype=np.float32)
    bf = lambda a: np.ascontiguousarray(np.asarray(a, dtype=np.float32)
                                        .astype(NPBF16))
    tokens = np.asarray(inputs["tokens"]).astype(np.int32)
    pos_emb = f(inputs["pos_emb"])
    lnpack = lambda a: np.ascontiguousarray(
        f(a)[:n_layers].reshape(n_layers, -1, 128).transpose(0, 2, 1))
    tri = np.tril(np.ones((128, 128), np.float32)).T  # tri[k,q] = k<=q

    # LN biases are folded as zeros (see kernel docstring)
    assert abs(f(inputs["ln1_b"])).max() == 0.0
    assert abs(f(inputs["ln2_b"])).max() == 0.0
    g1 = f(inputs["ln1_g"])[:n_layers]                 # [L, E]
    g2 = f(inputs["ln2_g"])[:n_layers]

    def wpack(w, gain=None):
        # [L, E, M] -> [L, 128, KT*M]; partition p holds rows k*128+p
        w = np.asarray(w, np.float32)[:n_layers]
        if gain is not None:
            w = w * gain[:, :, None]
        M = w.shape[-1]
        return np.ascontiguousarray(
            w.reshape(n_layers, KT, 128, M).transpose(0, 2, 1, 3)
            .reshape(n_layers, 128, KT * M).astype(NPBF16))

    w1 = np.asarray(inputs["W1"], np.float32)[:n_layers]   # [L, E, FF]
    w1 = w1 * g2[:, :, None]
    w1p = np.ascontiguousarray(
        w1.reshape(n_layers, KT, 128, 4, E).transpose(0, 3, 2, 1, 4)
        .reshape(n_layers, 4, 128, KT * E).astype(NPBF16))
    w2 = np.asarray(inputs["W2"], np.float32)[:n_layers]   # [L, FF, E]
    w2p = np.ascontiguousarray(
        w2.reshape(n_layers, 2, 16, 128, 2, 512).transpose(0, 1, 4, 3, 2, 5)
        .reshape(n_layers, 2, 2, 128, 16 * 512).astype(NPBF16))
    lw = np.asarray(inputs["lmh_w"], np.float32)           # [E, V]
    lwp = np.ascontiguousarray(
        lw.reshape(KT, 128, NVC, VCH).transpose(2, 1, 0, 3)
        .reshape(NVC, 128, KT * VCH).astype(NPBF16))

    common = {
        "tok_emb": bf(inputs["tok_emb"]),
        "Wq": wpack(inputs["Wq"], g1), "Wk": wpack(inputs["Wk"], g1),
        "Wv": wpack(inputs["Wv"], g1), "Wo": wpack(inputs["Wo"]),
        "W1": w1p, "W2": w2p,
        "b1": lnpack(inputs["b1"]), "b2": lnpack(inputs["b2"]),
        "lmhw": lwp,
        "lmhb": np.ascontiguousarray(
            np.broadcast_to(bf(inputs["lmh_b"])[None, :], (128, V))),
        "trid": np.ascontiguousarray(tri.astype(NPBF16)),
    }
    in_maps = []
    for c in range(NC):
        b, hf = c // 2, c % 2
        t0 = hf * RT
        toks = tokens[b, t0:t0 + RT]
        tok_idx = np.ascontiguousarray(toks.reshape(TT, 128).T)
        pT = pos_emb[t0:t0 + RT].T.astype(NPBF16)          # [E, RT]
        posTp = np.ascontiguousarray(
            pT.reshape(KT, 128, RT).transpose(1, 0, 2).reshape(128, KT * RT))
        pshard = 1 - hf
        # rows into agout_k viewed as [(s f), RT] (slot-major, E rows per
        # slot) and agout_v viewed as [(s p t), AUGW] (slot-major, 128*TT
        # rows per slot; partition p's tt blocks at rows p*TT + tt).
        gidx_np = np.stack([pshard * E + np.arange(128),
                            pshard * 128 * TT + np.arange(128) * TT],
                           axis=1).astype(np.int32)
        # partner keys: all allowed for hf=1 (keys before queries), all
        # masked for hf=0
        pbias_np = np.full((128, 1), 0.0 if hf else -1e30, np.float32)
        in_maps.append(dict(common, tok_idx=tok_idx, posT=posTp,
                            gidx=gidx_np, pbiasd=pbias_np))
    return in_maps


def kernel(**inputs):
    key = "nc"
    if key not in _cache:
        _cache[key] = _build()
    nc = _cache[key]
    in_maps = _prepare(inputs)
    res = run_bass_kernel_spmd(nc, in_maps, core_ids=list(range(NC)))
    out = np.empty((B, T, V), np.float32)
    for c in range(NC):
        b, hf = c // 2, c % 2
        out[b, hf * RT:(hf + 1) * RT] = np.asarray(
            res.results[c]["logits"]).astype(np.float32)
    return out


# revision 41
# speedup vs baseline: 1.2935x; 1.2935x over previous
"""Trainium2 Bass kernel for nn_Decoder_60198261621363.

6-layer dense transformer decoder (E=1024, H=16, FF=4096, V=32000, B=4, T=1024).

Sharding: data-parallel over tokens across 8 cores. Core c owns batch c//2,
token half c%2 (512 tokens). Weights are replicated (bf16); each layer does a
pairwise AllGather carrying that layer's K^T and V so the paired core sees the
other half's keys/values. Activations are feature-major (x^T: [E, tokens]);
the residual stream stays f32, all matmul operands are bf16.

Attention is two-pass to hide the AllGather: pass 1 computes own-half scores
and exps for ALL heads (suffix-restricted to the causal region, stored in a
persistent SBUF tile), pass 2 (first consumer of partner K/V) does partner
scores plus the full AV accumulation per head, with a 2-head score lookahead
so the partner-V readback is hidden too.  Own-half key chunks only need the
causal suffix of queries (plus a 128x128 triangular multiply on the diagonal
block); partner-half chunks are fully allowed or fully masked via a per-core
exp bias of 0 / -1e30.

LayerNorm: the gains are folded into the weights on the host (rows of
Wq/Wk/Wv scaled by ln1_g, W1 by ln2_g; ln biases are zero in this model), so
the kernel only computes hu = x - m on the h critical path and applies the
per-token 1/std at psum evacuation time (a column scale for feature-major
outputs; a per-partition scale via a transposed rT column for the token-major
V output).

All streamed weights are pre-transposed on the host into [128, k*free]
layouts so each partition reads one contiguous run per DMA.
"""
import sys

sys.path.insert(0, "/opt/trn_rl_repo")

import ml_dtypes
import numpy as np

import concourse.bass as bass
import concourse.mybir as mybir
import concourse.tile as tile
from concourse import bacc
from concourse.bass_utils import run_bass_kernel_spmd
from concourse.masks import make_identity

AF = mybir.ActivationFunctionType
ALU = mybir.AluOpType
F32 = mybir.dt.float32
F32R = mybir.dt.float32r
BF16 = mybir.dt.bfloat16
FP8 = mybir.dt.float8e4
I32 = mybir.dt.int32
NPBF16 = ml_dtypes.bfloat16

V, E, L, H, B, T = 32000, 1024, 6, 16, 4, 1024
HS = E // H          # 64
FF = 4 * E           # 4096
NC = 8               # cores
RT = 512             # tokens per core
KT = E // 128        # 8  E-tiles
TT = RT // 128       # 4  token-tiles per core
VCH = 1000           # lm-head column chunk (2 psum groups of 500)
NVC = V // VCH       # 32 chunks
SCALE = float(E) ** -0.5

# own-half score suffix: chunk kcc covers keys kcc*128..; only queries
# >= kcc*128 can see them.  eo column offsets pack the suffixes.
SUF = [RT - kcc * 128 for kcc in range(TT)]        # 512,384,256,128
EOFF = [0]
for w in SUF:
    EOFF.append(EOFF[-1] + w)
EOW = EOFF[-1]                                      # 1280

_cache = {}


def _build(n_layers=L, dbg=False):
    nc = bacc.Bacc("TRN2", target_bir_lowering=False, debug=False, num_devices=NC)
    dtensors = {}
    if dbg:
        for nm, shp, dt in [
            ("dbg_xemb", [128, KT, RT], F32), ("dbg_h1", [128, KT, RT], BF16),
            ("dbg_kown", [128, KT, RT], BF16), ("dbg_kpart", [128, KT, RT], BF16),
            ("dbg_vaug", [128, TT, H, HS + 1], BF16),
            ("dbg_vpaug", [128, TT, H, HS + 1], BF16),
            ("dbg_qT", [128, KT, RT], BF16),
            ("dbg_aT", [128, KT, RT], BF16), ("dbg_xl0", [128, KT, RT], F32),
        ]:
            dtensors[nm] = nc.dram_tensor(nm, shp, dt, kind="ExternalOutput")

    tok_emb = nc.dram_tensor("tok_emb", [V, E], BF16, kind="ExternalInput")
    tok_idx = nc.dram_tensor("tok_idx", [128, TT], I32, kind="ExternalInput")
    gidx = nc.dram_tensor("gidx", [128, 2], I32, kind="ExternalInput")
    posT = nc.dram_tensor("posT", [128, KT * RT], BF16, kind="ExternalInput")
    trid = nc.dram_tensor("trid", [128, 128], BF16, kind="ExternalInput")
    pbiasd = nc.dram_tensor("pbiasd", [128, 1], F32, kind="ExternalInput")
    Wq = nc.dram_tensor("Wq", [n_layers, 128, KT * E], BF16, kind="ExternalInput")
    Wk = nc.dram_tensor("Wk", [n_layers, 128, KT * E], BF16, kind="ExternalInput")
    Wv = nc.dram_tensor("Wv", [n_layers, 128, KT * E], BF16, kind="ExternalInput")
    Wo = nc.dram_tensor("Wo", [n_layers, 128, KT * E], BF16, kind="ExternalInput")
    W1 = nc.dram_tensor("W1", [n_layers, 4, 128, KT * E], BF16,
                        kind="ExternalInput")
    W2 = nc.dram_tensor("W2", [n_layers, 2, 2, 128, 16 * 512], BF16,
                        kind="ExternalInput")
    b1d = nc.dram_tensor("b1", [n_layers, 128, FF // 128], F32, kind="ExternalInput")
    b2d = nc.dram_tensor("b2", [n_layers, 128, KT], F32, kind="ExternalInput")
    lmhw = nc.dram_tensor("lmhw", [NVC, 128, KT * VCH], BF16, kind="ExternalInput")
    lmhb = nc.dram_tensor("lmhb", [128, V], BF16, kind="ExternalInput")
    logits = nc.dram_tensor("logits", [RT, V], BF16, kind="ExternalOutput")

    rgroups = [[0, 1], [2, 3], [4, 5], [6, 7]]
    AGK = E * RT        # elements of K^T in the AG payload
    AUGW = H * (HS + 1)                 # 1040 head-blocked V cols per tt
    AGV = TT * AUGW                     # per-partition V payload cols

    with tile.TileContext(nc) as tc:
        with (
            tc.tile_pool(name="persist", bufs=1) as pp,
            tc.tile_pool(name="sb", bufs=2) as sb,
            tc.tile_pool(name="wstream", bufs=2) as wp,
            tc.tile_pool(name="psm", bufs=3, space="PSUM") as psm,   # dense+scores
            tc.tile_pool(name="psa", bufs=3, space="PSUM") as psa,   # attn out
            tc.tile_pool(name="pst", bufs=1, space="PSUM") as pst,   # ln stats
            tc.tile_pool(name="dram", bufs=2, space="DRAM") as dram,
        ):
            # ---- persistent tiles
            x = pp.tile([128, KT, RT], F32R)       # residual stream x^T
            ones = pp.tile([128, 128], F32R)
            ident = pp.tile([128, 128], BF16)
            identf = pp.tile([128, 128], F32)
            make_identity(nc, identf[:])
            epsb = pp.tile([128, 1], F32)
            tri = pp.tile([128, 128], BF16)        # lower-tri (key<=query)
            pbias = pp.tile([128, 1], F32)
            nc.vector.memset(ones[:].bitcast(F32), 1.0)
            nc.vector.memset(epsb[:], 1e-5)
            make_identity(nc, ident[:])
            nc.sync.dma_start(tri[:], trid[:])
            nc.sync.dma_start(pbias[:], pbiasd[:])
            gidx_sb = pp.tile([128, 2], I32)
            nc.sync.dma_start(gidx_sb[:], gidx[:])

            v_aug = pp.tile([128, TT, H, HS + 1], BF16)   # own V, head-blocked
            vp_aug = pp.tile([128, TT, H, HS + 1], BF16)  # partner V
            nc.vector.memset(v_aug[:, :, :, HS:HS + 1], 1.0)
            nc.vector.memset(vp_aug[:, :, :, HS:HS + 1], 1.0)

            # pass-1 own-half attention accumulators (numerator rows 0-63
            # of each head block + denominator row 64), spilled from PSUM
            anum = pp.tile([128, H, RT], BF16)
            # zero-padded qT variants: qTz0 keeps even-head rows (0-63 of
            # each chunk), qTz1 keeps odd-head rows; the other half is zero
            # so score matmuls can use the full 128-row K^T block as a
            # dense-shaped stationary (FWL + weight-load pipelining).
            qTz0 = pp.tile([128, KT, RT], BF16)
            qTz1 = pp.tile([128, KT, RT], BF16)
            nc.vector.memset(qTz0[64:128, :, :], 0.0)
            nc.vector.memset(qTz1[0:64, :, :], 0.0)

            # ---- embedding gather + transpose + pos add
            idx = pp.tile([128, TT], I32)
            nc.sync.dma_start(idx[:], tok_idx[:])
            pos_sb = wp.tile([128, KT, RT], BF16, tag="wstream")
            nc.sync.dma_start(
                pos_sb[:], posT.ap().rearrange("p (k t) -> p k t", t=RT))
            for g in range(TT):
                emb = sb.tile([128, E], BF16, tag="emb", bufs=1)
                nc.gpsimd.indirect_dma_start(
                    out=emb[:], out_offset=None, in_=tok_emb[:],
                    in_offset=bass.IndirectOffsetOnAxis(
                        ap=idx[:, g:g + 1], axis=0),
                )
                for kt in range(KT):
                    tp = psm.tile([128, 128], BF16, space="PSUM", tag="psm")
                    nc.tensor.transpose(tp[:], emb[:, kt * 128:(kt + 1) * 128],
                                        ident[:])
                    nc.vector.tensor_add(
                        out=x[:, kt, g * 128:(g + 1) * 128],
                        in0=tp[:],
                        in1=pos_sb[:, kt, g * 128:(g + 1) * 128])

            if dbg:
                nc.sync.dma_start(dtensors["dbg_xemb"].ap(), x[:].bitcast(F32))

            def layer_norm():
                """Returns (hu, r, None): hu = x - mean (bf16, the matmul
                operand; LN gain is folded into the weights on the host),
                r = 1/std as a row-broadcast [128, RT] f32 (column scale for
                feature-major psum outputs)."""
                sum_ps = pst.tile([128, RT], F32, space="PSUM", tag="pst")
                sq_ps = pst.tile([128, RT], F32, space="PSUM", tag="pst2")
                for kt in range(KT):
                    sq = sb.tile([128, RT], F32R, tag="lnsq")
                    nc.scalar.activation(sq[:], x[:, kt, :], AF.Square)
                    nc.tensor.matmul(sum_ps[:], ones[:], x[:, kt, :],
                                     start=(kt == 0), stop=(kt == KT - 1))
                    nc.tensor.matmul(sq_ps[:], ones[:], sq[:],
                                     start=(kt == 0), stop=(kt == KT - 1))
                m = sb.tile([128, RT], F32, tag="ln_m", bufs=1)
                nc.vector.tensor_scalar_mul(m[:], sum_ps[:], 1.0 / E)
                hu = sb.tile([128, KT, RT], BF16, tag="h", bufs=1)
                for kt in range(KT):
                    nc.vector.tensor_sub(hu[:, kt, :], x[:, kt, :], m[:])
                mm = sb.tile([128, RT], F32, tag="ln_t")
                nc.vector.tensor_mul(mm[:], m[:], m[:])
                var = sb.tile([128, RT], F32, tag="ln_t")
                nc.vector.scalar_tensor_tensor(
                    var[:], sq_ps[:], 1.0 / E, mm[:], ALU.mult, ALU.subtract)
                std = sb.tile([128, RT], F32, tag="ln_t")
                nc.scalar.activation(std[:], var[:], AF.Sqrt, bias=epsb[:, 0:1])
                r = sb.tile([128, RT], F32, tag="ln_r", bufs=1)
                nc.vector.reciprocal_approx_fast(r[:], std[:])
                return hu, r, None

            def wload(src_ap, shape, tag="wstream"):
                wt = wp.tile([128] + shape, BF16, tag=tag)
                nc.sync.dma_start(
                    wt[:], src_ap.rearrange("p (k m) -> p k m", m=shape[-1]))
                return wt

            for li in range(n_layers):
                # ================= attention =================
                hu, r, _ = layer_norm()

                agin_k = dram.tile([AGK], BF16, tag="agin_k")
                agout_k = dram.tile([2, AGK], BF16, tag="agout_k")
                agin_v = dram.tile([128 * AGV], BF16, tag="agin_v")
                agout_v = dram.tile([2, 128 * AGV], BF16, tag="agout_v")

                # K^T own  [E, RT] as [128, KT, RT]; columns scaled by r
                kT_own = sb.tile([128, KT, RT], BF16, tag="kT_own", bufs=1)
                wt = wload(Wk[li], [KT, E])
                for mt in range(KT):
                    ps = psm.tile([128, RT], F32, space="PSUM", tag="psm")
                    for kt in range(KT):
                        nc.tensor.matmul(
                            ps[:], wt[:, kt, mt * 128:(mt + 1) * 128],
                            hu[:, kt, :],
                            start=(kt == 0), stop=(kt == KT - 1))
                    nc.vector.scalar_tensor_tensor(
                        kT_own[:, mt, :], ps[:], 0.0, r[:],
                        ALU.bypass, ALU.mult)

                nc.sync.dma_start(
                    agin_k[:].rearrange("(f t) -> f t", t=RT)
                    .rearrange("(k p) t -> p k t", p=128), kT_own[:])
                nc.gpsimd.collective_compute(
                    "AllGather", mybir.AluOpType.bypass,
                    replica_groups=rgroups,
                    ins=[agin_k[:].opt()], outs=[agout_k[:].opt()],
                )

                # rT for the token-major V evac (r is ready by now; placed
                # here so the transposes don't block the K matmuls)
                rT = sb.tile([128, TT], F32, tag="ln_rT", bufs=1)
                for tt in range(TT):
                    tp = psm.tile([128, 128], F32, space="PSUM", tag="psm")
                    nc.tensor.transpose(
                        tp[:], r[:, tt * 128:(tt + 1) * 128], identf[:])
                    nc.vector.tensor_copy(rT[:, tt:tt + 1], tp[:, 0:1])

                # V own, projected straight into the head-blocked aug layout;
                # rows (tokens) scaled by rT
                wt = wload(Wv[li], [KT, E])
                for tt in range(TT):
                    for mh in range(2):
                        ps = psm.tile([128, RT], F32, space="PSUM", tag="psm")
                        for kt in range(KT):
                            nc.tensor.matmul(
                                ps[:], hu[:, kt, tt * 128:(tt + 1) * 128],
                                wt[:, kt, mh * 512:(mh + 1) * 512],
                                start=(kt == 0), stop=(kt == KT - 1))
                        nc.vector.tensor_scalar_mul(
                            v_aug[:, tt, mh * 8:(mh + 1) * 8, 0:HS],
                            ps[:].rearrange("p (h f) -> p h f", f=HS),
                            rT[:, tt:tt + 1])

                # AG payload is the aug layout itself (ones column included)
                nc.sync.dma_start(
                    agin_v[:].rearrange("(p f) -> p f", p=128),
                    v_aug[:])
                nc.gpsimd.collective_compute(
                    "AllGather", mybir.AluOpType.bypass,
                    replica_groups=rgroups,
                    ins=[agin_v[:].opt()], outs=[agout_v[:].opt()],
                )

                # Q^T; columns scaled by r, split into the zero-padded
                # even/odd-head variants
                wt = wload(Wq[li], [KT, E])
                for mt in range(KT):
                    ps = psm.tile([128, RT], F32, space="PSUM", tag="psm")
                    for kt in range(KT):
                        nc.tensor.matmul(
                            ps[:], wt[:, kt, mt * 128:(mt + 1) * 128],
                            hu[:, kt, :],
                            start=(kt == 0), stop=(kt == KT - 1))
                    nc.vector.scalar_tensor_tensor(
                        qTz0[0:64, mt, :], ps[0:64, :], 0.0, r[0:64, :],
                        ALU.bypass, ALU.mult)
                    nc.vector.scalar_tensor_tensor(
                        qTz1[64:128, mt, :], ps[64:128, :], 0.0, r[64:128, :],
                        ALU.bypass, ALU.mult)

                # ---- pass 1: own-half attention for ALL heads: scores
                # (full 128-row K^T block as stationary, zero-padded qTz as
                # moving), exp (+ triangular mask on the diagonal block),
                # own-half AV accumulation, spilled to anum (numerator in
                # rows 0-63 of the head block, denominator in row 64).
                # Chunk kcc only affects queries >= kcc*128 (suffix).
                # Gives the AllGathers a long window to hide.
                for hd in range(H):
                    hp, sub = hd // 2, hd % 2
                    qz = qTz0 if sub == 0 else qTz1
                    es_ = []
                    for kcc in range(TT):
                        c0 = kcc * 128
                        sc = psm.tile([128, RT], F32, space="PSUM", tag="psm")
                        nc.tensor.matmul(
                            sc[:, c0:],
                            kT_own[:, hp, c0:c0 + 128],
                            qz[:, hp, c0:],
                            start=True, stop=True)
                        es = sb.tile([128, RT], BF16, tag="esc", bufs=12)
                        nc.scalar.activation(es[:, c0:], sc[:, c0:], AF.Exp,
                                             scale=SCALE)
                        nc.vector.tensor_mul(
                            es[:, c0:c0 + 128], es[:, c0:c0 + 128], tri[:])
                        es_.append(es)
                    av = psa.tile([128, RT], F32, space="PSUM", tag="psa")
                    for kcc in range(TT):
                        c0 = kcc * 128
                        nc.tensor.matmul(
                            av[0:HS + 1, c0:], v_aug[:, kcc, hd, :],
                            es_[kcc][:, c0:],
                            start=(kcc == 0), stop=(kcc == TT - 1))
                    nc.vector.tensor_copy(anum[0:HS + 1, hd, :],
                                          av[0:HS + 1, :])

                # partner K^T / V readback (depends on the AGs)
                kflat = agout_k[:].rearrange("s (f t) -> (s f) t", t=RT)
                # rows of AUGW so the indexed row length matches the
                # per-partition transfer length (as for K)
                vflat = agout_v[:].rearrange("s (p t f) -> (s p t) f",
                                             f=AUGW, t=TT)
                kT_part = sb.tile([128, KT, RT], BF16, tag="kT_part", bufs=1)
                for mt in range(KT):
                    nc.gpsimd.indirect_dma_start(
                        out=kT_part[:, mt, :], out_offset=None,
                        in_=kflat,
                        in_offset=bass.IndirectOffsetOnAxis(
                            ap=gidx_sb[:, 0:1], axis=0),
                        element_offset=mt * 128 * RT,
                    )
                # NOTE: the indirect offset is effectively scaled by the
                # dest AP's innermost contiguous run length, so the dest
                # must be flattened to rows of AUGW (matching gidx, which
                # holds pshard*512 + p*TT row indices).
                for tt in range(TT):
                    nc.gpsimd.indirect_dma_start(
                        out=vp_aug[:, tt, :, :].rearrange("p h f -> p (h f)"),
                        out_offset=None,
                        in_=vflat,
                        in_offset=bass.IndirectOffsetOnAxis(
                            ap=gidx_sb[:, 1:2], axis=0),
                        element_offset=tt * AUGW,
                    )

                # ---- pass 2: per head, partner scores (full width, exp
                # bias 0 / -1e30 per core), partner AV accumulation, then
                # combine with the pass-1 own-half accumulators.  2-head
                # score lookahead hides the partner-V readback.
                aT = sb.tile([128, KT, RT], BF16, tag="aT", bufs=1)
                p2eps = {}

                def p2_scores(hd):
                    hp, sub = hd // 2, hd % 2
                    qz = qTz0 if sub == 0 else qTz1
                    eps_ = []
                    for kcc in range(TT):
                        sc = psm.tile([128, RT], F32, space="PSUM", tag="psm")
                        nc.tensor.matmul(
                            sc[:],
                            kT_part[:, hp, kcc * 128:(kcc + 1) * 128],
                            qz[:, hp, :],
                            start=True, stop=True)
                        ep = sb.tile([128, RT], BF16, tag="esc", bufs=12)
                        nc.scalar.activation(ep[:], sc[:], AF.Exp,
                                             scale=SCALE, bias=pbias[:, 0:1])
                        eps_.append(ep)
                    p2eps[hd] = eps_

                def p2_avs(hd):
                    hp, lo = hd // 2, (hd % 2) * 64
                    eps_ = p2eps.pop(hd)
                    av = psa.tile([128, RT], F32, space="PSUM", tag="psa")
                    for kcc in range(TT):
                        nc.tensor.matmul(
                            av[0:HS + 1, :], vp_aug[:, kcc, hd, :],
                            eps_[kcc][:],
                            start=(kcc == 0), stop=(kcc == TT - 1))
                    num = sb.tile([64, RT], F32, tag="numt", bufs=2)
                    nc.vector.tensor_add(num[:], av[0:HS, :],
                                         anum[0:HS, hd, :])
                    dtmp = sb.tile([1, RT], F32, tag="dtmp", bufs=2)
                    nc.vector.tensor_add(dtmp[0:1, :], av[HS:HS + 1, :],
                                         anum[HS:HS + 1, hd, :])
                    nc.vector.reciprocal_approx_fast(dtmp[0:1, :],
                                                     dtmp[0:1, :])
                    rtmp = sb.tile([64, RT], F32, tag="rtmp", bufs=2)
                    nc.gpsimd.partition_broadcast(rtmp[:], dtmp[0:1, :])
                    nc.vector.tensor_mul(aT[lo:lo + 64, hp, :], num[:],
                                         rtmp[:])

                LOOK = 2
                for hd in range(H + LOOK):
                    if hd < H:
                        p2_scores(hd)
                    if hd >= LOOK:
                        p2_avs(hd - LOOK)

                if dbg and li == 0:
                    nc.sync.dma_start(dtensors["dbg_h1"].ap(), hu[:])
                    nc.sync.dma_start(dtensors["dbg_kown"].ap(), kT_own[:])
                    nc.sync.dma_start(dtensors["dbg_kpart"].ap(), kT_part[:])
                    nc.sync.dma_start(dtensors["dbg_vaug"].ap(), v_aug[:])
                    nc.sync.dma_start(dtensors["dbg_vpaug"].ap(), vp_aug[:])
                    nc.sync.dma_start(dtensors["dbg_qT"].ap(), qTz0[:])
                    nc.sync.dma_start(dtensors["dbg_aT"].ap(), aT[:])

                # ---- Wo + residual
                wt = wload(Wo[li], [KT, E])
                for mt in range(KT):
                    ps = psm.tile([128, RT], F32, space="PSUM", tag="psm")
                    for kt in range(KT):
                        nc.tensor.matmul(
                            ps[:], wt[:, kt, mt * 128:(mt + 1) * 128],
                            aT[:, kt, :],
                            start=(kt == 0), stop=(kt == KT - 1))
                    nc.vector.tensor_add(x[:, mt, :], x[:, mt, :], ps[:])

                # ================= FFN (two halves of FF) =================
                hu2, r2, _ = layer_norm()
                b1t = sb.tile([128, FF // 128], F32, tag="b1t")
                nc.sync.dma_start(b1t[:], b1d[li])
                b2t = sb.tile([128, KT], F32, tag="b2t")
                nc.sync.dma_start(b2t[:], b2d[li])
                for fh in range(2):
                    up = sb.tile([128, 16, RT], BF16, tag="up", bufs=1)
                    for c in range(2):
                        wt = wload(W1[li, fh * 2 + c], [KT, E])
                        for mt in range(KT):
                            ps = psm.tile([128, RT], F32, space="PSUM", tag="psm")
                            for kt in range(KT):
                                nc.tensor.matmul(
                                    ps[:], wt[:, kt, mt * 128:(mt + 1) * 128],
                                    hu2[:, kt, :],
                                    start=(kt == 0), stop=(kt == KT - 1))
                            # scale by 1/std in place, then biased relu
                            nc.vector.scalar_tensor_tensor(
                                ps[:], ps[:], 0.0, r2[:], ALU.bypass, ALU.mult)
                            gft = (fh * 2 + c) * 8 + mt
                            nc.scalar.activation(up[:, c * 8 + mt, :], ps[:],
                                                 AF.Relu, bias=b1t[:, gft:gft + 1])
                    for m2 in range(2):
                        wt = wload(W2[li, fh, m2], [16, 512])
                        for mt in range(4):
                            ps = psm.tile([128, RT], F32, space="PSUM", tag="psm")
                            for kt in range(16):
                                nc.tensor.matmul(
                                    ps[:], wt[:, kt, mt * 128:(mt + 1) * 128],
                                    up[:, kt, :],
                                    start=(kt == 0), stop=(kt == 15))
                            ft = m2 * 4 + mt
                            if fh == 1:
                                nc.vector.scalar_tensor_tensor(
                                    x[:, ft, :], ps[:], b2t[:, ft:ft + 1],
                                    x[:, ft, :],
                                    ALU.add, ALU.add)
                            else:
                                nc.vector.tensor_add(x[:, ft, :], x[:, ft, :],
                                                     ps[:])
                if dbg and li == 0:
                    nc.sync.dma_start(dtensors["dbg_xl0"].ap(), x[:].bitcast(F32))

            # ================= LM head =================
            xb = sb.tile([128, KT, RT], BF16, tag="h", bufs=1)
            for kt in range(KT):
                nc.vector.tensor_copy(xb[:, kt, :], x[:, kt, :])
            for vc in range(NVC):
                wt = wload(lmhw[vc], [KT, VCH])
                bb = sb.tile([128, VCH], BF16, tag="bb")
                nc.sync.dma_start(bb[:], lmhb[:, vc * VCH:(vc + 1) * VCH])
                for tt in range(TT):
                    lg = sb.tile([128, VCH], BF16, tag="lg", bufs=3)
                    for hv in range(2):
                        ps = psm.tile([128, VCH // 2], F32, space="PSUM",
                                      tag="psm")
                        for kt in range(KT):
                            nc.tensor.matmul(
                                ps[:], xb[:, kt, tt * 128:(tt + 1) * 128],
                                wt[:, kt, hv * 500:(hv + 1) * 500],
                                start=(kt == 0), stop=(kt == KT - 1))
                        nc.vector.tensor_add(
                            lg[:, hv * 500:(hv + 1) * 500], ps[:],
                            bb[:, hv * 500:(hv + 1) * 500])
                    nc.sync.dma_start(
                        logits.ap().rearrange("(t p) v -> p t v", p=128)
                        [:, tt, vc * VCH:(vc + 1) * VCH], lg[:])
    nc.compile()
    return nc


def _prepare(inputs, n_layers=L):
    """Build the 8 per-core input maps from full inputs."""
    f = lambda a: np.ascontiguousarray(np.asarray(a), dtype=np.float32)
    bf = lambda a: np.ascontiguousarray(np.asarray(a, dtype=np.float32)
                                        .astype(NPBF16))
    tokens = np.asarray(inputs["tokens"]).astype(np.int32)
    pos_emb = f(inputs["pos_emb"])
    lnpack = lambda a: np.ascontiguousarray(
        f(a)[:n_layers].reshape(n_layers, -1, 128).transpose(0, 2, 1))
    tri = np.tril(np.ones((128, 128), np.float32)).T  # tri[k,q] = k<=q

    # LN biases are folded as zeros (see kernel docstring)
    assert abs(f(inputs["ln1_b"])).max() == 0.0
    assert abs(f(inputs["ln2_b"])).max() == 0.0
    g1 = f(inputs["ln1_g"])[:n_layers]                 # [L, E]
    g2 = f(inputs["ln2_g"])[:n_layers]

    def wpack(w, gain=None):
        # [L, E, M] -> [L, 128, KT*M]; partition p holds rows k*128+p
        w = np.asarray(w, np.float32)[:n_layers]
        if gain is not None:
            w = w * gain[:, :, None]
        M = w.shape[-1]
        return np.ascontiguousarray(
            w.reshape(n_layers, KT, 128, M).transpose(0, 2, 1, 3)
            .reshape(n_layers, 128, KT * M).astype(NPBF16))

    w1 = np.asarray(inputs["W1"], np.float32)[:n_layers]   # [L, E, FF]
    w1 = w1 * g2[:, :, None]
    w1p = np.ascontiguousarray(
        w1.reshape(n_layers, KT, 128, 4, E).transpose(0, 3, 2, 1, 4)
        .reshape(n_layers, 4, 128, KT * E).astype(NPBF16))
    w2 = np.asarray(inputs["W2"], np.float32)[:n_layers]   # [L, FF, E]
    w2p = np.ascontiguousarray(
        w2.reshape(n_layers, 2, 16, 128, 2, 512).transpose(0, 1, 4, 3, 2, 5)
        .reshape(n_layers, 2, 2, 128, 16 * 512).astype(NPBF16))
    lw = np.asarray(inputs["lmh_w"], np.float32)           # [E, V]
    lwp = np.ascontiguousarray(
        lw.reshape(KT, 128, NVC, VCH).transpose(2, 1, 0, 3)
        .reshape(NVC, 128, KT * VCH).astype(NPBF16))

    common = {
        "tok_emb": bf(inputs["tok_emb"]),
        "Wq": wpack(inputs["Wq"], g1), "Wk": wpack(inputs["Wk"], g1),
        "Wv": wpack(inputs["Wv"], g1), "Wo": wpack(inputs["Wo"]),
        "W1": w1p, "W2": w2p,
        "b1": lnpack(inputs["b1"]), "b2": lnpack(inputs["b2"]),
        "lmhw": lwp,
        "lmhb": np.ascontiguousarray(
            np.broadcast_to(bf(inputs["lmh_b"])[None, :], (128, V))),
        "trid": np.ascontiguousarray(tri.astype(NPBF16)),
    }
    in_maps = []
    for c in range(NC):
        b, hf = c // 2, c % 2
        t0 = hf * RT
        toks = tokens[b, t0:t0 + RT]
        tok_idx = np.ascontiguousarray(toks.reshape(TT, 128).T)
        pT = pos_emb[t0:t0 + RT].T.astype(NPBF16)          # [E, RT]
        posTp = np.ascontiguousarray(
            pT.reshape(KT, 128, RT).transpose(1, 0, 2).reshape(128, KT * RT))
        pshard = 1 - hf
        # rows into agout_k viewed as [(s f), RT] (slot-major, E rows per
        # slot) and agout_v viewed as [(s p t), AUGW] (slot-major, 128*TT
        # rows per slot; partition p's tt blocks at rows p*TT + tt).
        gidx_np = np.stack([pshard * E + np.arange(128),
                            pshard * 128 * TT + np.arange(128) * TT],
                           axis=1).astype(np.int32)
        # partner keys: all allowed for hf=1 (keys before queries), all
        # masked for hf=0
        pbias_np = np.full((128, 1), 0.0 if hf else -1e30, np.float32)
        in_maps.append(dict(common, tok_idx=tok_idx, posT=posTp,
                            gidx=gidx_np, pbiasd=pbias_np))
    return in_maps


def kernel(**inputs):
    key = "nc"
    if key not in _cache:
        _cache[key] = _build()
    nc = _cache[key]
    in_maps = _prepare(inputs)
    res = run_bass_kernel_spmd(nc, in_maps, core_ids=list(range(NC)))
    out = np.empty((B, T, V), np.float32)
    for c in range(NC):
        b, hf = c // 2, c % 2
        out[b, hf * RT:(hf + 1) * RT] = np.asarray(
            res.results[c]["logits"]).astype(np.float32)
    return out
